# revision 44
# baseline (speedup 1.0000x reference)
"""Causal multi-head attention (B=2, S=2048, H=1024, 16 heads, hd=64) on 8
Trainium2 NeuronCores.

Sharding: batch x head-group. Core c handles batch c//4 and the 4 heads
4*(c%4)..4*(c%4)+3 (a 256-wide column slice of Q/K/V). Each core computes its
heads' contribution to the output projection (row-parallel Wo); the host sums
the 4 partials per batch and adds bo.

v2 design (ACT-bound attention, ladder schedule):
  - xt DMA is quarter-major (tokens ascending), so the kernel runs a ladder:
    q/k mc0 + v for token quarter nb, then attention slots in ASCENDING qb
    with pair1 (heads 2/3) delayed one step. Everything later (q/k mc1, v
    tail, outproj) is emitted as ~1-3.5us filler units woven between score
    windows, where the PE has slack while ACT grinds exps.
  - Score matmuls for the head PAIR are interleaved: head A's kT lives on
    partitions 0-63, head B's on 64-127, so consecutive MMs occupy disjoint
    PE row groups and different PSUM banks -> the hardware runs them
    concurrently (measured 109.6ns vs 215.2ns per MM). Scores cost ~half.
  - Softmax: exp on ACT (scale=1/8 folded; scores bounded so no max-sub);
    denominator via a ones column in vaug (row 64 of ctx PSUM). Epilogue:
    raw denominator rows copied to SBUF, broadcast across 64 partitions
    each by two concurrent col-tiled K=1 matmuls, then reciprocal as
    exp(-ln d) on ACT over all 128 lanes (a DVE InstReciprocal on [1,512]
    is single-lane and costs 3.4us - avoided), then DVE muls write
    normalized ctxT in the outproj stationary layout.
  - PSUM: pss pool 3x[128,2,512] (6 banks: score windows + all fillers +
    broadcast borrow), psc 2x[128,512] (ctx for head A/B of the live slot).
  - DMA: per (family, engine) stream moves only ~9GB/s (1KB packet per
    ~110ns serially), so startup-critical pieces are partition-split and
    spread across the sync + gpsimd queue families (scalar/vector boot too
    late to help; vector cannot trigger DMAs at all). wq/wk are mc-major
    on the host so the prefix loads only the mc0 half with 2KB runs.
    Output chunks split 2-4 ways, alternating families, to cut tail drain.
"""
import ml_dtypes
import numpy as np

import concourse.bass as bass
import concourse.mybir as mybir
import concourse.tile as tile
from concourse.bass import ts
from concourse.bass_utils import run_bass_kernel_spmd

B, S, H, NH, HD = 2, 2048, 1024, 16, 64
NCORES = 8
HPC = 4            # heads per core
HSW = HPC * HD     # 256: head-slice width
F32 = mybir.dt.float32
BF16 = mybir.dt.bfloat16
NEG = -1.0e9
NQB = S // 512      # 4 query blocks
NTC = S // 128      # 16 token chunks
EXP = mybir.ActivationFunctionType.Exp


def _split_multi_waits(nc) -> int:
    """walrus accepts at most ONE sync wait per instruction. Split any
    multi-wait instruction into single-wait NOPs (same engine, just before
    it) + the instruction carrying the last wait."""
    n = 0
    for f in nc.m.functions:
        for blk in f.blocks:
            new_insts = []
            for inst in blk.instructions:
                si = inst.sync_info
                if si is not None and si.on_wait and len(si.on_wait) > 1:
                    waits = list(si.on_wait)
                    for i, w in enumerate(waits[:-1]):
                        new_insts.append(mybir.InstNoOp(
                            name=f"{inst.name}-ws{i}",
                            engine=inst.engine,
                            bass_nofuse=True,
                            sync_info=mybir.SyncInfo(on_wait=[w], on_update=[]),
                        ))
                        n += 1
                    si.on_wait = [waits[-1]]
                new_insts.append(inst)
            blk.instructions[:] = new_insts
    return n


def _build():
    nc = bass.Bass()
    xt_d = nc.dram_tensor("xt", [H, S], BF16, kind="ExternalInput")
    wq_d = nc.dram_tensor("wq", [128, 2, 8, 128], BF16, kind="ExternalInput")
    wk_d = nc.dram_tensor("wk", [128, 2, 8, 128], BF16, kind="ExternalInput")
    wv_d = nc.dram_tensor("wv", [128, 8, HSW], BF16, kind="ExternalInput")
    wo_d = nc.dram_tensor("wo", [128, 2, H], BF16, kind="ExternalInput")
    vb_d = nc.dram_tensor("vb", [128, HSW], F32, kind="ExternalInput")
    bqkvt_d = nc.dram_tensor("bqkvt", [128, 2, 2], F32, kind="ExternalInput")
    mb_d = nc.dram_tensor("mb", [128, 128], F32, kind="ExternalInput")
    out_d = nc.dram_tensor("out", [S, H], BF16, kind="ExternalOutput")

    with tile.TileContext(nc) as tc:
        with tc.tile_pool(name="const", bufs=1) as constp, \
             tc.tile_pool(name="persist", bufs=1) as pers:
            wq = constp.tile([128, 2, 8, 128], BF16)
            wk = constp.tile([128, 2, 8, 128], BF16)
            wv = constp.tile([128, 8, HSW], BF16)
            wo = constp.tile([128, 2, H], BF16)
            vb = constp.tile([128, HSW], F32)      # v bias, row-replicated
            bqkvt = constp.tile([128, 2, 2], F32)  # [p, mc, q|k] per-row bias
            mbt = constp.tile([128, 128], F32)
            onesf = constp.tile([128, 512], F32)
            nc.vector.memset(onesf, 1.0)
            onesb = constp.tile([1, 128], BF16)
            nc.vector.tensor_copy(out=onesb, in_=onesf[0:1, 0:128])

            xt = pers.tile([128, 8, S], BF16)     # resident for whole kernel
            qT = pers.tile([128, 2, S], BF16)     # [2 heads x 64 hd, mc, tok]
            kT = pers.tile([128, 2, S], BF16)
            vaug = pers.tile([128, 4, NTC, HD + 1], BF16)
            ctxT = pers.tile([128, 2, S], BF16)   # outproj stationary layout
            nc.vector.tensor_copy(
                out=vaug[:, :, :, HD:HD + 1],
                in_=onesf[:, 0:64].rearrange("p (a b o) -> p a b o", a=4, b=16))

            # ---- DMA schedule: quarter-major xt so token quarters land in
            # ascending order; weights for the early passes first. ----
            # The prefix is gated by wq/wk mc0 + xt quarter 0. Weights
            # are mc-major on the host so the mc0 half DMAs as contiguous
            # 2KB runs; sync-family first (it boots earliest), gpsimd takes
            # the odd xt pieces and the second-wave weights.
            for ph in range(4):
                p0 = ph * 32
                nc.sync.dma_start(out=wq[p0:p0 + 32, 0, :, :],
                                  in_=wq_d[p0:p0 + 32, 0, :, :])
                nc.sync.dma_start(out=wk[p0:p0 + 32, 0, :, :],
                                  in_=wk_d[p0:p0 + 32, 0, :, :])
            nc.sync.dma_start(out=bqkvt, in_=bqkvt_d[:, :, :])
            for kc in range(8):
                if kc < 2:
                    # first chunks gate the whole prefix: finest split
                    for ph in range(4):
                        p0 = kc * 128 + ph * 32
                        eng = nc.sync if ph < 2 else nc.gpsimd
                        eng.dma_start(out=xt[ph * 32:ph * 32 + 32, kc, 0:512],
                                      in_=xt_d[p0:p0 + 32, 0:512])
                else:
                    nc.sync.dma_start(
                        out=xt[0:64, kc, 0:512],
                        in_=xt_d[kc * 128:kc * 128 + 64, 0:512])
                    nc.gpsimd.dma_start(
                        out=xt[64:128, kc, 0:512],
                        in_=xt_d[kc * 128 + 64:kc * 128 + 128, 0:512])
            nc.gpsimd.dma_start(out=wv[0:64, :, :], in_=wv_d[0:64, :, :])
            nc.gpsimd.dma_start(out=wv[64:128, :, :], in_=wv_d[64:128, :, :])
            nc.scalar.dma_start(out=vb, in_=vb_d[:, :])
            nc.scalar.dma_start(out=mbt, in_=mb_d[:, :])
            nc.gpsimd.dma_start(out=wq[0:64, 1, :, :], in_=wq_d[0:64, 1, :, :])
            nc.gpsimd.dma_start(out=wq[64:128, 1, :, :],
                                in_=wq_d[64:128, 1, :, :])
            nc.gpsimd.dma_start(out=wk[0:64, 1, :, :], in_=wk_d[0:64, 1, :, :])
            nc.gpsimd.dma_start(out=wk[64:128, 1, :, :],
                                in_=wk_d[64:128, 1, :, :])
            for nb in range(1, 4):
                for kc in range(8):
                    nc.sync.dma_start(out=xt[:, kc, ts(nb, 512)],
                                      in_=xt_d[ts(kc, 128), ts(nb, 512)])
                if nb == 1:
                    nc.gpsimd.dma_start(out=wo[:, 0, :], in_=wo_d[:, 0, :])
                    nc.gpsimd.dma_start(out=wo[:, 1, :], in_=wo_d[:, 1, :])

            with tc.tile_pool(name="pss", bufs=3, space="PSUM") as pss, \
                 tc.tile_pool(name="psc", bufs=2, space="PSUM") as psc, \
                 tc.tile_pool(name="attnp", bufs=4) as attnp, \
                 tc.tile_pool(name="outp", bufs=4) as outp, \
                 tc.tile_pool(name="smallp", bufs=2) as smallp:

                def qk_unit(w, brow, dst, mc, nb):
                    """8-MM projection chunk: dst[:, mc, nb-block] ="""
                    def run():
                        ps = pss.tile([128, 2, 512], F32, tag="s", name="qk")
                        for kc in range(8):
                            nc.tensor.matmul(ps[:, 0, :],
                                             w[:, mc, kc, :],
                                             xt[:, kc, ts(nb, 512)],
                                             start=(kc == 0), stop=(kc == 7))
                        nc.vector.tensor_scalar_add(
                            out=dst[:, mc, ts(nb, 512)], in0=ps[:, 0, :],
                            scalar1=bqkvt[:, mc, brow:brow + 1])
                    return run

                def v_unit(tt):
                    """v projection for token chunks 2tt, 2tt+1."""
                    def run():
                        ps = pss.tile([128, 2, 512], F32, tag="s", name="vps")
                        for u in range(2):
                            t = 2 * tt + u
                            for kc in range(8):
                                nc.tensor.matmul(ps[:, u, 0:HSW],
                                                 xt[:, kc, ts(t, 128)],
                                                 wv[:, kc, :],
                                                 start=(kc == 0), stop=(kc == 7))
                        for u in range(2):
                            t = 2 * tt + u
                            nc.vector.tensor_add(
                                vaug[:, :, t, 0:HD],
                                ps[:, u, 0:HSW].rearrange(
                                    "p (h d) -> p h d", h=4),
                                vb.rearrange("p (h d) -> p h d", h=4))
                    return run

                QK_COST, V_COST, OP_COST = 2100, 2000, 1100

                def outproj_unit(t):
                    """out_partial[t-chunk, 1024] = ctxT.T @ Wo-slice."""
                    def run():
                        ps = pss.tile([128, 2, 512], F32, tag="s", name="ops")
                        for n2 in range(2):
                            for mc in range(2):
                                nc.tensor.matmul(ps[:, n2, :],
                                                 ctxT[:, mc, ts(t, 128)],
                                                 wo[:, mc, ts(n2, 512)],
                                                 start=(mc == 0), stop=(mc == 1))
                        osb = outp.tile([128, H], BF16, tag="osb", name="osb")
                        nc.vector.tensor_copy(
                            out=osb.rearrange("p (a b) -> p a b", a=2),
                            in_=ps)
                        nsp = 4 if t >= 8 else 2
                        for ph in range(nsp):
                            w = 128 // nsp
                            eng = nc.sync if (t + ph) % 2 == 0 else nc.gpsimd
                            eng.dma_start(
                                out=out_d[t * 128 + ph * w:
                                          t * 128 + (ph + 1) * w, :],
                                in_=osb[ph * w:(ph + 1) * w, :])
                    return run

                units = []
                ucosts = []
                for nb in range(1, 4):
                    units.append(qk_unit(wq, 0, qT, 0, nb))
                    units.append(qk_unit(wk, 1, kT, 0, nb))
                    units.append(qk_unit(wq, 0, qT, 1, nb - 1))
                    units.append(qk_unit(wk, 1, kT, 1, nb - 1))
                    ucosts += [QK_COST, QK_COST, QK_COST, QK_COST]
                units.append(qk_unit(wq, 0, qT, 1, 3))
                units.append(qk_unit(wk, 1, kT, 1, 3))
                ucosts += [QK_COST, QK_COST]
                # units consumed before slot X must be <= req[X]:
                #   slot order: (0,0) (1,0) (0,1) (2,0) (1,1) (3,0) (2,1) (3,1)
                # (v units are NOT here: slot (qb,0) emits its own v units
                # inside its first windows, overlapping the exps)
                reqs = {(1, 0): 2, (0, 1): 4, (2, 0): 6, (1, 1): 8,
                        (3, 0): 10, (2, 1): 12, (3, 1): 14}
                emitted = [0]
                credit = [0.0]
                deferred = []   # outproj units, appended as qb completes

                def force_units(n):
                    while emitted[0] < n:
                        units[emitted[0]]()
                        emitted[0] += 1
                        credit[0] = 0.0

                def pop_units():
                    # spend accumulated ACT-PE deficit on filler units
                    while True:
                        if emitted[0] < len(units):
                            cost = ucosts[emitted[0]]
                            if credit[0] < cost:
                                return
                            units[emitted[0]]()
                            emitted[0] += 1
                            credit[0] -= cost
                        elif deferred:
                            if credit[0] < OP_COST:
                                return
                            deferred.pop(0)()
                            credit[0] -= OP_COST
                        else:
                            return

                finishers = []   # deferred epilogue tails (Ln/Exp/muls)

                def attention_slot(qb, pair):
                    """One head-pair slot: heads (2*pair, 2*pair+1), query
                    block qb. Paired scores via row-group interleave; one
                    PSUM window per k-tile holds BOTH heads (u=0/1, adjacent
                    banks -> concurrent row-group matmuls, single exp op)."""
                    mc = pair
                    T = 4 * qb + 4           # k-tiles
                    cA = psc.tile([128, 512], F32, tag="ctx", name="cA")
                    cB = psc.tile([128, 512], F32, tag="ctx", name="cB")
                    qlo, qhi = qb * 512, (qb + 1) * 512
                    pend = []   # [(et, kb, lo)] not yet consumed by ctx

                    def emit_ctx(p):
                        (et, kb, lo) = p
                        nc.tensor.matmul(cA[0:HD + 1, lo:512],
                                         vaug[:, 2 * mc, kb, :],
                                         et[:, 0, lo:512],
                                         start=(kb == 0), stop=(kb == T - 1),
                                         skip_group_check=True)
                        nc.tensor.matmul(cB[0:HD + 1, lo:512],
                                         vaug[:, 2 * mc + 1, kb, :],
                                         et[:, 1, lo:512],
                                         start=(kb == 0), stop=(kb == T - 1),
                                         skip_group_check=True)

                    for kb in range(T):
                        j = kb - 4 * qb
                        lo = 128 * j if j > 0 else 0
                        sps = pss.tile([128, 2, 512], F32, tag="s", name="sps")
                        nc.tensor.matmul(
                            sps[:, 0, lo:512],
                            kT[0:HD, mc, ts(kb, 128)],
                            qT[0:HD, mc, qlo + lo:qhi],
                            start=True, stop=True)
                        nc.tensor.matmul(
                            sps[:, 1, lo:512],
                            kT[HD:128, mc, ts(kb, 128)],
                            qT[HD:128, mc, qlo + lo:qhi],
                            start=True, stop=True)
                        if j >= 0:
                            for u in range(2):
                                nc.vector.tensor_add(
                                    sps[:, u, 128 * j:128 * j + 128],
                                    sps[:, u, 128 * j:128 * j + 128],
                                    mbt)
                        et = attnp.tile([128, 2, 512], BF16, tag="et",
                                        name="et")
                        nc.scalar.activation(out=et[:, :, lo:512],
                                             in_=sps[:, :, lo:512],
                                             func=EXP, scale=0.125)
                        pend.append((et, kb, lo))
                        if kb == 0 and finishers:
                            finishers.pop(0)()
                        if pair == 0 and kb < 2:
                            # this slot's v projections, woven under the exps
                            v_unit(2 * qb + kb)()
                            credit[0] = 0.0
                        if len(pend) > 2:
                            emit_ctx(pend.pop(0))
                        # act-pe deficit for this window, overpumped 2x:
                        # ACT has plenty of idle, so emitting fillers early
                        # is safe while late forced batches starve it
                        n = 512 - lo
                        credit[0] += 2 * ((2 * n + 352) / 1.2 + 150
                                          - (n * 0.43 + 170)
                                          - (n * 0.85 + 110))
                        # hold fillers during the last two windows: their
                        # DVE work (1.2us outproj casts) otherwise queues
                        # AHEAD of the epilogue's denominator copies and
                        # stalls the broadcast matmuls ~2us every slot
                        if kb < T - 2:
                            pop_units()
                    for p in pend:
                        emit_ctx(p)
                    # epilogue part 1 (inline): copy raw denominators (row
                    # HD) to SBUF and broadcast them across 64 partitions
                    # each with two col-tiled K=1 matmuls (concurrent)
                    dsb = smallp.tile([1, 2, 512], BF16, tag="dsb", name="dsb")
                    nc.vector.tensor_copy(out=dsb[:, 0, :],
                                          in_=cA[HD:HD + 1, :])
                    nc.vector.tensor_copy(out=dsb[:, 1, :],
                                          in_=cB[HD:HD + 1, :])
                    bps = pss.tile([128, 2, 512], F32, tag="s", name="bps")
                    nc.tensor.matmul(bps[0:HD, 0, :], onesb[:, 0:HD],
                                     dsb[:, 0, :], start=True, stop=True)
                    nc.tensor.matmul(bps[HD:128, 0, :], onesb[:, HD:128],
                                     dsb[:, 1, :], start=True, stop=True)
                    pop_units()

                    def finish():
                        # part 2 (deferred past the next slot's first
                        # window so ACT bridges the chain latency with a
                        # useful exp): reciprocal as exp(-ln d) on ACT over
                        # all 128 lanes, then DVE muls into ctxT. (A DVE
                        # InstReciprocal on [1,512] costs 3.4us - avoid.)
                        lnb = smallp.tile([128, 512], F32, tag="lnb",
                                          name="lnb")
                        nc.scalar.activation(
                            out=lnb, in_=bps[:, 0, :],
                            func=mybir.ActivationFunctionType.Ln)
                        bsb = smallp.tile([128, 512], BF16, tag="bsb",
                                          name="bsb")
                        nc.scalar.activation(out=bsb, in_=lnb, func=EXP,
                                             scale=-1.0)
                        nc.vector.tensor_mul(
                            out=ctxT[0:HD, mc, ts(qb, 512)],
                            in0=cA[0:HD, :], in1=bsb[0:HD, :])
                        nc.vector.tensor_mul(
                            out=ctxT[HD:128, mc, ts(qb, 512)],
                            in0=cB[0:HD, :], in1=bsb[HD:128, :])
                    finishers.append(finish)

                # ---- prefix: quarter 0 q/k (v weaves into slot (0,0)) ----
                qk_unit(wq, 0, qT, 0, 0)()
                qk_unit(wk, 1, kT, 0, 0)()

                # ---- ladder ----
                for (qb, pair) in [(0, 0), (1, 0), (0, 1), (2, 0), (1, 1),
                                   (3, 0), (3, 1), (2, 1)]:
                    force_units(reqs.get((qb, pair), 0))
                    attention_slot(qb, pair)
                    if pair == 1:
                        for t in range(4 * qb, 4 * qb + 4):
                            deferred.append(outproj_unit(t))
                # tail: remaining fillers + outproj of late blocks
                while finishers:
                    finishers.pop(0)()
                force_units(len(units))
                while deferred:
                    deferred.pop(0)()

    _split_multi_waits(nc)
    return nc


_NC_CACHE = []


def _get_nc():
    if not _NC_CACHE:
        _NC_CACHE.append(_build())
    return _NC_CACHE[0]


def _triangle_mask() -> np.ndarray:
    """mbt[p, f] = 0 where p <= f (key p attends to query f), else NEG."""
    p = np.arange(128)[:, None]
    f = np.arange(128)[None, :]
    return np.where(p <= f, 0.0, NEG).astype(np.float32)


def _in_maps(inputs: dict) -> list[dict]:
    bf16 = ml_dtypes.bfloat16
    x = np.asarray(inputs["hidden_states"], dtype=np.float32).astype(bf16)
    Wq = np.asarray(inputs["Wq"], dtype=np.float32).astype(bf16)
    Wk = np.asarray(inputs["Wk"], dtype=np.float32).astype(bf16)
    Wv = np.asarray(inputs["Wv"], dtype=np.float32).astype(bf16)
    Wo = np.asarray(inputs["Wo"], dtype=np.float32).astype(bf16)

    xts = [np.ascontiguousarray(x[b].T) for b in range(B)]
    mbt = _triangle_mask()

    def wlayout(wt, c):
        # [c*128, n] -> [128, c, n] so per-partition DMA runs are contiguous
        return np.ascontiguousarray(
            wt.reshape(c, 128, wt.shape[1]).transpose(1, 0, 2))

    def wlayout_mc(wt):
        # [1024, 256] -> [128, 2(mc), 8(kc), 128]: mc-major so the prefix
        # DMAs just the mc0 half as contiguous 2KB per-partition runs
        return np.ascontiguousarray(
            wt.reshape(8, 128, 2, 128).transpose(1, 2, 0, 3))

    bqf = np.asarray(inputs["bq"], dtype=np.float32)
    bkf = np.asarray(inputs["bk"], dtype=np.float32)
    bvf = np.asarray(inputs["bv"], dtype=np.float32)
    maps = []
    for c in range(NCORES):
        b, hg = c // 4, c % 4
        hs = slice(hg * HSW, (hg + 1) * HSW)
        # [p, mc, q|k] fp32 per-row bias for the DVE tensor_scalar add
        bqkvt = np.ascontiguousarray(
            np.stack([bqf[hs].reshape(2, 128), bkf[hs].reshape(2, 128)],
                     axis=-1).transpose(1, 0, 2))
        maps.append({
            "xt": xts[b],
            "wq": wlayout_mc(np.ascontiguousarray(Wq[hs, :].T)),
            "wk": wlayout_mc(np.ascontiguousarray(Wk[hs, :].T)),
            "wv": wlayout(np.ascontiguousarray(Wv[hs, :].T), 8),
            "wo": wlayout(np.ascontiguousarray(Wo[:, hs].T), 2),
            "vb": np.ascontiguousarray(
                np.broadcast_to(bvf[hs][None, :], (128, HSW))),
            "bqkvt": bqkvt,
            "mb": mbt,
        })
    return maps


def run(inputs: dict, **spmd_kwargs):
    """Returns (full_output, BassKernelResults)."""
    nc = _get_nc()
    res = run_bass_kernel_spmd(nc, _in_maps(inputs), list(range(NCORES)),
                               **spmd_kwargs)
    bo = np.asarray(inputs["bo"], dtype=np.float32)
    out = np.empty((B, S, H), dtype=np.float32)
    for b in range(B):
        acc = res.results[4 * b]["out"].astype(np.float32)
        for hg in range(1, 4):
            acc = acc + res.results[4 * b + hg]["out"].astype(np.float32)
        out[b] = acc + bo
    return out, res


def kernel(**inputs) -> np.ndarray:
    out, _ = run(inputs)
    return out


# revision 45
# speedup vs baseline: 1.2098x; 1.2098x over previous
"""Causal multi-head attention (B=2, S=2048, H=1024, 16 heads, hd=64) on 8
Trainium2 NeuronCores.

Sharding: batch x head-group. Core c handles batch c//4 and the 4 heads
4*(c%4)..4*(c%4)+3 (a 256-wide column slice of Q/K/V). Each core computes its
heads' contribution to the output projection (row-parallel Wo); the host sums
the 4 partials per batch and adds bo.

v2 design (ACT-bound attention, ladder schedule):
  - xt DMA is quarter-major (tokens ascending), so the kernel runs a ladder:
    q/k mc0 + v for token quarter nb, then attention slots in ASCENDING qb
    with pair1 (heads 2/3) delayed one step. Everything later (q/k mc1, v
    tail, outproj) is emitted as ~1-3.5us filler units woven between score
    windows, where the PE has slack while ACT grinds exps.
  - Score matmuls for the head PAIR are interleaved: head A's kT lives on
    partitions 0-63, head B's on 64-127, so consecutive MMs occupy disjoint
    PE row groups and different PSUM banks -> the hardware runs them
    concurrently (measured 109.6ns vs 215.2ns per MM). Scores cost ~half.
  - Softmax: exp on ACT (scale=1/8 folded; scores bounded so no max-sub);
    denominator via a ones column in vaug (row 64 of ctx PSUM). Epilogue:
    raw denominator rows copied to SBUF, broadcast across 64 partitions
    each by two concurrent col-tiled K=1 matmuls, then reciprocal as
    exp(-ln d) on ACT over all 128 lanes (a DVE InstReciprocal on [1,512]
    is single-lane and costs 3.4us - avoided), then DVE muls write
    normalized ctxT in the outproj stationary layout.
  - PSUM: pss pool 3x[128,2,512] (6 banks: score windows + all fillers +
    broadcast borrow), psc 2x[128,512] (ctx for head A/B of the live slot).
  - DMA: per (family, engine) stream moves only ~9GB/s (1KB packet per
    ~110ns serially), so startup-critical pieces are partition-split and
    spread across the sync + gpsimd queue families (scalar/vector boot too
    late to help; vector cannot trigger DMAs at all). wq/wk are mc-major
    on the host so the prefix loads only the mc0 half with 2KB runs.
    Output chunks split 2-4 ways, alternating families, to cut tail drain.
"""
import ml_dtypes
import numpy as np

import concourse.bass as bass
import concourse.mybir as mybir
import concourse.tile as tile
from concourse.bass import ts
from concourse.bass_utils import run_bass_kernel_spmd

B, S, H, NH, HD = 2, 2048, 1024, 16, 64
NCORES = 8
HPC = 4            # heads per core
HSW = HPC * HD     # 256: head-slice width
F32 = mybir.dt.float32
BF16 = mybir.dt.bfloat16
NEG = -1.0e9
NQB = S // 512      # 4 query blocks
NTC = S // 128      # 16 token chunks
EXP = mybir.ActivationFunctionType.Exp


def _split_multi_waits(nc) -> int:
    """walrus accepts at most ONE sync wait per instruction. Split any
    multi-wait instruction into single-wait NOPs (same engine, just before
    it) + the instruction carrying the last wait."""
    n = 0
    for f in nc.m.functions:
        for blk in f.blocks:
            new_insts = []
            for inst in blk.instructions:
                si = inst.sync_info
                if si is not None and si.on_wait and len(si.on_wait) > 1:
                    waits = list(si.on_wait)
                    for i, w in enumerate(waits[:-1]):
                        new_insts.append(mybir.InstNoOp(
                            name=f"{inst.name}-ws{i}",
                            engine=inst.engine,
                            bass_nofuse=True,
                            sync_info=mybir.SyncInfo(on_wait=[w], on_update=[]),
                        ))
                        n += 1
                    si.on_wait = [waits[-1]]
                new_insts.append(inst)
            blk.instructions[:] = new_insts
    return n


def _build():
    nc = bass.Bass()
    xt_d = nc.dram_tensor("xt", [H, S], BF16, kind="ExternalInput")
    wq_d = nc.dram_tensor("wq", [128, 2, 8, 128], BF16, kind="ExternalInput")
    wk_d = nc.dram_tensor("wk", [128, 2, 8, 128], BF16, kind="ExternalInput")
    wv_d = nc.dram_tensor("wv", [128, 8, HSW], BF16, kind="ExternalInput")
    wo_d = nc.dram_tensor("wo", [128, 2, H], BF16, kind="ExternalInput")
    vb_d = nc.dram_tensor("vb", [128, HSW], F32, kind="ExternalInput")
    bqkvt_d = nc.dram_tensor("bqkvt", [128, 2, 2], F32, kind="ExternalInput")
    mb_d = nc.dram_tensor("mb", [128, 128], F32, kind="ExternalInput")
    out_d = nc.dram_tensor("out", [S, H], BF16, kind="ExternalOutput")

    with tile.TileContext(nc) as tc:
        with tc.tile_pool(name="const", bufs=1) as constp, \
             tc.tile_pool(name="persist", bufs=1) as pers:
            wq = constp.tile([128, 2, 8, 128], BF16)
            wk = constp.tile([128, 2, 8, 128], BF16)
            wv = constp.tile([128, 8, HSW], BF16)
            wo = constp.tile([128, 2, H], BF16)
            vb = constp.tile([128, HSW], F32)      # v bias, row-replicated
            bqkvt = constp.tile([128, 2, 2], F32)  # [p, mc, q|k] per-row bias
            mbt = constp.tile([128, 128], F32)
            onesf = constp.tile([128, 512], F32)
            nc.vector.memset(onesf, 1.0)
            onesb = constp.tile([1, 128], BF16)
            nc.vector.tensor_copy(out=onesb, in_=onesf[0:1, 0:128])

            xt = pers.tile([128, 8, S], BF16)     # resident for whole kernel
            qT = pers.tile([128, 2, S], BF16)     # [2 heads x 64 hd, mc, tok]
            kT = pers.tile([128, 2, S], BF16)
            vaug = pers.tile([128, 4, NTC, HD + 1], BF16)
            ctxT = pers.tile([128, 2, S], BF16)   # outproj stationary layout
            nc.vector.tensor_copy(
                out=vaug[:, :, :, HD:HD + 1],
                in_=onesf[:, 0:64].rearrange("p (a b o) -> p a b o", a=4, b=16))

            # ---- DMA schedule: quarter-major xt so token quarters land in
            # ascending order; weights for the early passes first. ----
            # The prefix is gated by wq/wk mc0 + xt quarter 0. Weights
            # are mc-major on the host so the mc0 half DMAs as contiguous
            # 2KB runs; sync-family first (it boots earliest), gpsimd takes
            # the odd xt pieces and the second-wave weights.
            for ph in range(4):
                p0 = ph * 32
                nc.sync.dma_start(out=wq[p0:p0 + 32, 0, :, :],
                                  in_=wq_d[p0:p0 + 32, 0, :, :])
                nc.sync.dma_start(out=wk[p0:p0 + 32, 0, :, :],
                                  in_=wk_d[p0:p0 + 32, 0, :, :])
            nc.sync.dma_start(out=bqkvt, in_=bqkvt_d[:, :, :])
            for kc in range(8):
                if kc < 2:
                    # first chunks gate the whole prefix: finest split
                    for ph in range(4):
                        p0 = kc * 128 + ph * 32
                        eng = nc.sync if ph < 2 else nc.gpsimd
                        eng.dma_start(out=xt[ph * 32:ph * 32 + 32, kc, 0:512],
                                      in_=xt_d[p0:p0 + 32, 0:512])
                else:
                    nc.sync.dma_start(
                        out=xt[0:64, kc, 0:512],
                        in_=xt_d[kc * 128:kc * 128 + 64, 0:512])
                    nc.gpsimd.dma_start(
                        out=xt[64:128, kc, 0:512],
                        in_=xt_d[kc * 128 + 64:kc * 128 + 128, 0:512])
            nc.gpsimd.dma_start(out=wv[0:64, :, :], in_=wv_d[0:64, :, :])
            nc.gpsimd.dma_start(out=wv[64:128, :, :], in_=wv_d[64:128, :, :])
            nc.scalar.dma_start(out=vb, in_=vb_d[:, :])
            nc.scalar.dma_start(out=mbt, in_=mb_d[:, :])
            nc.gpsimd.dma_start(out=wq[0:64, 1, :, :], in_=wq_d[0:64, 1, :, :])
            nc.gpsimd.dma_start(out=wq[64:128, 1, :, :],
                                in_=wq_d[64:128, 1, :, :])
            nc.gpsimd.dma_start(out=wk[0:64, 1, :, :], in_=wk_d[0:64, 1, :, :])
            nc.gpsimd.dma_start(out=wk[64:128, 1, :, :],
                                in_=wk_d[64:128, 1, :, :])
            for nb in range(1, 4):
                for kc in range(8):
                    nc.sync.dma_start(out=xt[:, kc, ts(nb, 512)],
                                      in_=xt_d[ts(kc, 128), ts(nb, 512)])
                if nb == 1:
                    nc.gpsimd.dma_start(out=wo[:, 0, :], in_=wo_d[:, 0, :])
                    nc.gpsimd.dma_start(out=wo[:, 1, :], in_=wo_d[:, 1, :])

            with tc.tile_pool(name="pss", bufs=3, space="PSUM") as pss, \
                 tc.tile_pool(name="psc", bufs=2, space="PSUM") as psc, \
                 tc.tile_pool(name="attnp", bufs=4) as attnp, \
                 tc.tile_pool(name="outp", bufs=4) as outp, \
                 tc.tile_pool(name="smallp", bufs=2) as smallp:

                def qk_unit(w, brow, dst, mc, nb):
                    """8-MM projection chunk: dst[:, mc, nb-block] ="""
                    def run():
                        ps = pss.tile([128, 2, 512], F32, tag="s", name="qk")
                        for kc in range(8):
                            nc.tensor.matmul(ps[:, 0, :],
                                             w[:, mc, kc, :],
                                             xt[:, kc, ts(nb, 512)],
                                             start=(kc == 0), stop=(kc == 7))
                        nc.vector.tensor_scalar_add(
                            out=dst[:, mc, ts(nb, 512)], in0=ps[:, 0, :],
                            scalar1=bqkvt[:, mc, brow:brow + 1])
                    return run

                def v_unit(tt):
                    """v projection for token chunks 2tt, 2tt+1."""
                    def run():
                        ps = pss.tile([128, 2, 512], F32, tag="s", name="vps")
                        for u in range(2):
                            t = 2 * tt + u
                            for kc in range(8):
                                nc.tensor.matmul(ps[:, u, 0:HSW],
                                                 xt[:, kc, ts(t, 128)],
                                                 wv[:, kc, :],
                                                 start=(kc == 0), stop=(kc == 7))
                        for u in range(2):
                            t = 2 * tt + u
                            nc.vector.tensor_add(
                                vaug[:, :, t, 0:HD],
                                ps[:, u, 0:HSW].rearrange(
                                    "p (h d) -> p h d", h=4),
                                vb.rearrange("p (h d) -> p h d", h=4))
                    return run

                QK_COST, V_COST, OP_COST = 2100, 2000, 1100

                def outproj_unit(t):
                    """out_partial[t-chunk, 1024] = ctxT.T @ Wo-slice."""
                    def run():
                        ps = pss.tile([128, 2, 512], F32, tag="s", name="ops")
                        for n2 in range(2):
                            for mc in range(2):
                                nc.tensor.matmul(ps[:, n2, :],
                                                 ctxT[:, mc, ts(t, 128)],
                                                 wo[:, mc, ts(n2, 512)],
                                                 start=(mc == 0), stop=(mc == 1))
                        osb = outp.tile([128, H], BF16, tag="osb", name="osb")
                        nc.vector.tensor_copy(
                            out=osb.rearrange("p (a b) -> p a b", a=2),
                            in_=ps)
                        nsp = 4 if t >= 8 else 2
                        for ph in range(nsp):
                            w = 128 // nsp
                            eng = nc.sync if (t + ph) % 2 == 0 else nc.gpsimd
                            eng.dma_start(
                                out=out_d[t * 128 + ph * w:
                                          t * 128 + (ph + 1) * w, :],
                                in_=osb[ph * w:(ph + 1) * w, :])
                    return run

                units = []
                ucosts = []
                for nb in range(1, 4):
                    units.append(qk_unit(wq, 0, qT, 0, nb))
                    units.append(qk_unit(wk, 1, kT, 0, nb))
                    units.append(qk_unit(wq, 0, qT, 1, nb - 1))
                    units.append(qk_unit(wk, 1, kT, 1, nb - 1))
                    ucosts += [QK_COST, QK_COST, QK_COST, QK_COST]
                units.append(qk_unit(wq, 0, qT, 1, 3))
                units.append(qk_unit(wk, 1, kT, 1, 3))
                ucosts += [QK_COST, QK_COST]
                # units consumed before slot X must be <= req[X]:
                #   slot order: (0,0) (1,0) (0,1) (2,0) (1,1) (3,0) (2,1) (3,1)
                # (v units are NOT here: slot (qb,0) emits its own v units
                # inside its first windows, overlapping the exps)
                reqs = {(1, 0): 2, (0, 1): 4, (2, 0): 6, (1, 1): 8,
                        (3, 0): 10, (2, 1): 12, (3, 1): 14}
                emitted = [0]
                credit = [0.0]
                deferred = []   # outproj units, appended as qb completes

                def force_units(n):
                    while emitted[0] < n:
                        units[emitted[0]]()
                        emitted[0] += 1
                        credit[0] = 0.0

                def pop_units():
                    # spend accumulated ACT-PE deficit on filler units
                    while True:
                        if emitted[0] < len(units):
                            cost = ucosts[emitted[0]]
                            if credit[0] < cost:
                                return
                            units[emitted[0]]()
                            emitted[0] += 1
                            credit[0] -= cost
                        elif deferred:
                            if credit[0] < OP_COST:
                                return
                            deferred.pop(0)()
                            credit[0] -= OP_COST
                        else:
                            return

                finishers = []   # deferred epilogue tails (Ln/Exp/muls)

                def attention_slot(qb, pair):
                    """One head-pair slot: heads (2*pair, 2*pair+1), query
                    block qb. Paired scores via row-group interleave; one
                    PSUM window per k-tile holds BOTH heads (u=0/1, adjacent
                    banks -> concurrent row-group matmuls, single exp op)."""
                    mc = pair
                    T = 4 * qb + 4           # k-tiles
                    cA = psc.tile([128, 512], F32, tag="ctx", name="cA")
                    cB = psc.tile([128, 512], F32, tag="ctx", name="cB")
                    qlo, qhi = qb * 512, (qb + 1) * 512
                    pend = []   # [(et, kb, lo)] not yet consumed by ctx

                    def emit_ctx(p):
                        (et, kb, lo) = p
                        nc.tensor.matmul(cA[0:HD + 1, lo:512],
                                         vaug[:, 2 * mc, kb, :],
                                         et[:, 0, lo:512],
                                         start=(kb == 0), stop=(kb == T - 1),
                                         skip_group_check=True)
                        nc.tensor.matmul(cB[0:HD + 1, lo:512],
                                         vaug[:, 2 * mc + 1, kb, :],
                                         et[:, 1, lo:512],
                                         start=(kb == 0), stop=(kb == T - 1),
                                         skip_group_check=True)

                    for kb in range(T):
                        j = kb - 4 * qb
                        lo = 128 * j if j > 0 else 0
                        sps = pss.tile([128, 2, 512], F32, tag="s", name="sps")
                        nc.tensor.matmul(
                            sps[:, 0, lo:512],
                            kT[0:HD, mc, ts(kb, 128)],
                            qT[0:HD, mc, qlo + lo:qhi],
                            start=True, stop=True)
                        nc.tensor.matmul(
                            sps[:, 1, lo:512],
                            kT[HD:128, mc, ts(kb, 128)],
                            qT[HD:128, mc, qlo + lo:qhi],
                            start=True, stop=True)
                        if j >= 0:
                            for u in range(2):
                                nc.vector.tensor_add(
                                    sps[:, u, 128 * j:128 * j + 128],
                                    sps[:, u, 128 * j:128 * j + 128],
                                    mbt)
                        et = attnp.tile([128, 2, 512], BF16, tag="et",
                                        name="et")
                        nc.scalar.activation(out=et[:, :, lo:512],
                                             in_=sps[:, :, lo:512],
                                             func=EXP, scale=0.125)
                        pend.append((et, kb, lo))
                        if kb == 0 and finishers:
                            finishers.pop(0)()
                        if pair == 0 and kb < 2:
                            # this slot's v projections, woven under the exps
                            v_unit(2 * qb + kb)()
                            credit[0] = 0.0
                        if len(pend) > 2:
                            emit_ctx(pend.pop(0))
                        # act-pe deficit for this window, overpumped 2x:
                        # ACT has plenty of idle, so emitting fillers early
                        # is safe while late forced batches starve it
                        n = 512 - lo
                        credit[0] += 2 * ((2 * n + 352) / 1.2 + 150
                                          - (n * 0.43 + 170)
                                          - (n * 0.85 + 110))
                        pop_units()
                    for p in pend:
                        emit_ctx(p)
                    # epilogue part 1 (inline): copy raw denominators (row
                    # HD) to SBUF and broadcast them across 64 partitions
                    # each with two col-tiled K=1 matmuls (concurrent)
                    dsb = smallp.tile([1, 2, 512], BF16, tag="dsb", name="dsb")
                    nc.vector.tensor_copy(out=dsb[:, 0, :],
                                          in_=cA[HD:HD + 1, :])
                    nc.vector.tensor_copy(out=dsb[:, 1, :],
                                          in_=cB[HD:HD + 1, :])
                    bps = pss.tile([128, 2, 512], F32, tag="s", name="bps")
                    nc.tensor.matmul(bps[0:HD, 0, :], onesb[:, 0:HD],
                                     dsb[:, 0, :], start=True, stop=True)
                    nc.tensor.matmul(bps[HD:128, 0, :], onesb[:, HD:128],
                                     dsb[:, 1, :], start=True, stop=True)

                    def finish():
                        # part 2 (deferred past the next slot's first
                        # window so ACT bridges the chain latency with a
                        # useful exp): reciprocal as exp(-ln d) on ACT over
                        # all 128 lanes, then DVE muls into ctxT. (A DVE
                        # InstReciprocal on [1,512] costs 3.4us - avoid.)
                        lnb = smallp.tile([128, 512], F32, tag="lnb",
                                          name="lnb")
                        nc.scalar.activation(
                            out=lnb, in_=bps[:, 0, :],
                            func=mybir.ActivationFunctionType.Ln)
                        bsb = smallp.tile([128, 512], BF16, tag="bsb",
                                          name="bsb")
                        nc.scalar.activation(out=bsb, in_=lnb, func=EXP,
                                             scale=-1.0)
                        nc.vector.tensor_mul(
                            out=ctxT[0:HD, mc, ts(qb, 512)],
                            in0=cA[0:HD, :], in1=bsb[0:HD, :])
                        nc.vector.tensor_mul(
                            out=ctxT[HD:128, mc, ts(qb, 512)],
                            in0=cB[0:HD, :], in1=bsb[HD:128, :])
                    finishers.append(finish)

                # ---- prefix: quarter 0 q/k (v weaves into slot (0,0)) ----
                qk_unit(wq, 0, qT, 0, 0)()
                qk_unit(wk, 1, kT, 0, 0)()

                # ---- ladder ----
                for (qb, pair) in [(0, 0), (1, 0), (0, 1), (2, 0), (1, 1),
                                   (3, 0), (3, 1), (2, 1)]:
                    force_units(reqs.get((qb, pair), 0))
                    attention_slot(qb, pair)
                    if pair == 1:
                        for t in range(4 * qb, 4 * qb + 4):
                            deferred.append(outproj_unit(t))
                # tail: remaining fillers + outproj of late blocks
                while finishers:
                    finishers.pop(0)()
                force_units(len(units))
                while deferred:
                    deferred.pop(0)()

    _split_multi_waits(nc)
    return nc


_NC_CACHE = []


def _get_nc():
    if not _NC_CACHE:
        _NC_CACHE.append(_build())
    return _NC_CACHE[0]


def _triangle_mask() -> np.ndarray:
    """mbt[p, f] = 0 where p <= f (key p attends to query f), else NEG."""
    p = np.arange(128)[:, None]
    f = np.arange(128)[None, :]
    return np.where(p <= f, 0.0, NEG).astype(np.float32)


def _in_maps(inputs: dict) -> list[dict]:
    bf16 = ml_dtypes.bfloat16
    x = np.asarray(inputs["hidden_states"], dtype=np.float32).astype(bf16)
    Wq = np.asarray(inputs["Wq"], dtype=np.float32).astype(bf16)
    Wk = np.asarray(inputs["Wk"], dtype=np.float32).astype(bf16)
    Wv = np.asarray(inputs["Wv"], dtype=np.float32).astype(bf16)
    Wo = np.asarray(inputs["Wo"], dtype=np.float32).astype(bf16)

    xts = [np.ascontiguousarray(x[b].T) for b in range(B)]
    mbt = _triangle_mask()

    def wlayout(wt, c):
        # [c*128, n] -> [128, c, n] so per-partition DMA runs are contiguous
        return np.ascontiguousarray(
            wt.reshape(c, 128, wt.shape[1]).transpose(1, 0, 2))

    def wlayout_mc(wt):
        # [1024, 256] -> [128, 2(mc), 8(kc), 128]: mc-major so the prefix
        # DMAs just the mc0 half as contiguous 2KB per-partition runs
        return np.ascontiguousarray(
            wt.reshape(8, 128, 2, 128).transpose(1, 2, 0, 3))

    bqf = np.asarray(inputs["bq"], dtype=np.float32)
    bkf = np.asarray(inputs["bk"], dtype=np.float32)
    bvf = np.asarray(inputs["bv"], dtype=np.float32)
    maps = []
    for c in range(NCORES):
        b, hg = c // 4, c % 4
        hs = slice(hg * HSW, (hg + 1) * HSW)
        # [p, mc, q|k] fp32 per-row bias for the DVE tensor_scalar add
        bqkvt = np.ascontiguousarray(
            np.stack([bqf[hs].reshape(2, 128), bkf[hs].reshape(2, 128)],
                     axis=-1).transpose(1, 0, 2))
        maps.append({
            "xt": xts[b],
            "wq": wlayout_mc(np.ascontiguousarray(Wq[hs, :].T)),
            "wk": wlayout_mc(np.ascontiguousarray(Wk[hs, :].T)),
            "wv": wlayout(np.ascontiguousarray(Wv[hs, :].T), 8),
            "wo": wlayout(np.ascontiguousarray(Wo[:, hs].T), 2),
            "vb": np.ascontiguousarray(
                np.broadcast_to(bvf[hs][None, :], (128, HSW))),
            "bqkvt": bqkvt,
            "mb": mbt,
        })
    return maps


def run(inputs: dict, **spmd_kwargs):
    """Returns (full_output, BassKernelResults)."""
    nc = _get_nc()
    res = run_bass_kernel_spmd(nc, _in_maps(inputs), list(range(NCORES)),
                               **spmd_kwargs)
    bo = np.asarray(inputs["bo"], dtype=np.float32)
    out = np.empty((B, S, H), dtype=np.float32)
    for b in range(B):
        acc = res.results[4 * b]["out"].astype(np.float32)
        for hg in range(1, 4):
            acc = acc + res.results[4 * b + hg]["out"].astype(np.float32)
        out[b] = acc + bo
    return out, res


def kernel(**inputs) -> np.ndarray:
    out, _ = run(inputs)
    return out
